# revision 36
# baseline (speedup 1.0000x reference)
"""Trainium2 Bass kernel for nn_CrossAttentionModule (sparse cross-attention).

Strategy
--------
Data-parallel over batch: 32 batches -> 8 NeuronCores, 4 batches each. No
collectives. Per batch, everything is computed in "layout B" (keys on SBUF
partitions, query positions on the free dim):

  scores^T[k, n] = sum_d kp[d, k] * qp[d, n]     (PE, row-tiled: 4 heads packed)
  e^T = exp(0.25 * scores^T + maskneg[k])        (ScalarE, mask folded into the
                                                  per-partition bias; one pass)
  ctx^T[d, n]   = sum_k [vp | 1][k, d] * e^T     (PE, col-tiled 4 heads; the
                                                  ones column yields the softmax
                                                  denominator for free)
  r_h = 1 / (denom_h + eps)                      (DVE reciprocal_approx_fast)
  attended^T    = (ctx^T * r) @ wo + bo          (PE)
  max_attn[n]   = max_k sum_h e_h^T[k,n] * r_h[n]/4
                                                 (DVE muls/adds with PE-broadcast
                                                  r tiles; final partition max via
                                                  PE transpose + free-dim reduce)

Sparsity: the key mask is known on the host, so keys/values are compacted
(gathered) on the host to the max unmasked count (padded to 128, pad columns
get -1e9 mask bias -> exp == 0). This roughly halves K for every engine.

All-masked batches fall out naturally: e == 0 -> denom == 0 -> r = 1/eps,
0 * huge == 0 for both outputs.
"""

import math
import sys

import numpy as np

for _p in ("/opt/trn_rl_repo",):
    if _p not in sys.path:
        sys.path.insert(0, _p)

B, N, D_MODEL = 32, 512, 64
K_FULL = 1024
H, DK = 4, 16
NCORES = 8
NB = B // NCORES  # batches per core
NEG = -1.0e9
EPS = 1e-30

_CACHE = {}
LAST_EXEC_NS = None
LAST_TRACE_DIR = None
LAST_RESULT = None


def _build(nk: int):
    """Build the SPMD Bass graph for one core processing NB batches with
    nk key-tiles of 128 (KP = 128*nk compacted keys)."""
    import concourse.bass as bass
    import concourse.bacc as bacc
    import concourse.mybir as mybir
    from concourse.tile import TileContext

    f32 = mybir.dt.float32
    bf16 = mybir.dt.bfloat16
    AF = mybir.ActivationFunctionType
    AX = mybir.AxisListType

    KP = 128 * nk
    NC2 = N // 128  # n-chunks of 128 (=4)

    nc = bacc.Bacc()
    qT = nc.declare_dram_parameter("qT", [NB, 65, N], bf16, isOutput=False)
    kT = nc.declare_dram_parameter("kT", [NB, 65, KP], bf16, isOutput=False)
    vT = nc.declare_dram_parameter("vT", [NB, 65, KP], bf16, isOutput=False)
    mneg = nc.declare_dram_parameter("mneg", [NB, 128, nk], f32, isOutput=False)
    cpack = nc.declare_dram_parameter("cpack", [128, 1096], bf16, isOutput=False)
    att = nc.declare_dram_parameter("att", [NB, N, 64], f32, isOutput=True)
    mx = nc.declare_dram_parameter("mx", [NB, N], f32, isOutput=True)

    with TileContext(nc) as tc:
        with (
            tc.tile_pool(name="consts", bufs=1) as cpool,
            tc.tile_pool(name="io", bufs=2) as iop,
            tc.tile_pool(name="proj", bufs=2) as projp,
            tc.tile_pool(name="epool", bufs=3) as epool,
            tc.tile_pool(name="work", bufs=3) as workp,
            tc.tile_pool(name="spsum", bufs=2, space="PSUM") as spsum,
            tc.tile_pool(name="cpsum", bufs=2, space="PSUM") as cpsum,
            tc.tile_pool(name="ppsum", bufs=2, space="PSUM") as ppsum,
        ):
            cp = cpool.tile_from(cpack[:, :])
            # column layout: wqb(64) wkb(64) wvo(68) wob(64) bob(64)
            #                sel(4) bsel(512) selb(128) iden(128)
            wqb_s = cp[0:65, 0:64]
            wkb_s = cp[0:65, 64:128]
            wvo_s = cp[0:65, 128:196]
            wob_s = cp[:, 196:260]
            bob_s = cp[0:65, 260:324]
            sel_s = cp[:, 324:328]
            bsel_s = cp[0:4, 328:840]
            selb_s = cp[0:4, 840:968]
            iden_s = cp[:, 968:1096]

            # ---- load all batches once ----------------------------------
            qTbs, kTbs, vTbs, mnegbs = [], [], [], []
            for b in range(NB):
                eng = nc.sync if b == 0 else nc.gpsimd
                mnegb = iop.tile([128, nk], f32, tag="mnegb", name="mnegb", bufs=NB)
                eng.dma_start(out=mnegb[:, :], in_=mneg[b])
                qTb = iop.tile([65, N], bf16, tag="qTb", name="qTb", bufs=NB)
                eng.dma_start(out=qTb[:, :], in_=qT[b])
                kTb = iop.tile([65, KP], bf16, tag="kTb", name="kTb", bufs=NB)
                eng.dma_start(out=kTb[:, :], in_=kT[b])
                vTb = iop.tile([65, KP], bf16, tag="vTb", name="vTb", bufs=NB)
                eng.dma_start(out=vTb[:, :], in_=vT[b])
                qTbs.append(qTb); kTbs.append(kTb); vTbs.append(vTb); mnegbs.append(mnegb)


            state = {}

            def proj(b):
                """Q/K/V projections (emitted mid-k-loop of the previous
                batch, when scores-psum slots are free)."""
                qTo, kTo, vTo = qTbs[b], kTbs[b], vTbs[b]
                qp_ps = spsum.tile([128, 512], f32, tag="sc", name="qp_ps")
                for h in range(H):
                    nc.tensor.matmul(
                        qp_ps[32 * h : 32 * h + 16, :],
                        lhsT=wqb_s[:, 16 * h : 16 * h + 16],
                        rhs=qTo[:, :],
                        start=True, stop=True,
                        tile_position=(0, 32 * h),
                    )
                qp = projp.tile([128, N], bf16, tag="qp", name="qp")
                nc.scalar.copy(qp[:, :], qp_ps[:, :])

                kp = projp.tile([128, KP], bf16, tag="kp", name="kp")
                kp_ps = spsum.tile([128, 1024], f32, tag="sc", name="kp_ps")
                for c0 in range(0, KP, 512):
                    w = min(512, KP - c0)
                    for h in range(H):
                        nc.tensor.matmul(
                            kp_ps[32 * h : 32 * h + 16, c0 : c0 + w],
                            lhsT=wkb_s[:, 16 * h : 16 * h + 16],
                            rhs=kTo[:, c0 : c0 + w],
                            start=True, stop=True,
                            tile_position=(0, 32 * h),
                        )
                nc.scalar.copy(kp[:, :], kp_ps[:, :KP])

                vpo = projp.tile([128, 68 * nk], bf16, tag="vpo", name="vpo")
                vp_ps = spsum.tile([128, 68 * nk], f32, tag="sc", name="vp_ps")
                for t in range(nk):
                    nc.tensor.matmul(
                        vp_ps[:, 68 * t : 68 * (t + 1)],
                        lhsT=vTo[:, 128 * t : 128 * (t + 1)],
                        rhs=wvo_s[:, :],
                        start=True, stop=True,
                    )
                nc.scalar.copy(vpo[:, :], vp_ps[:, :])
                state[b] = dict(qp=qp, kp=kp, vpo=vpo)

            def ph1(b):
                """Scores/exp/context k-loop (PE+ACT heavy)."""
                qTo, mnegS = qTbs[b], mnegbs[b]
                st = state[b]
                qp, kp, vpo = st["qp"], st["kp"], st["vpo"]
                # ctx^T accumulator: head h rows 32h..32h+15, denom row 32h+16.
                ctx_ps = cpsum.tile([128, 512], f32, tag="cq", name="ctx_ps")

                # Ones matmul: writes 1.0 everywhere, clears + sets the whole
                # bank's has_written bits so the interleaved head groups can
                # accumulate with start=False in any order; the 1.0 is
                # subtracted in the ctxs evacuation (bias=-1).
                nc.tensor.matmul(
                    ctx_ps[:, :],
                    lhsT=qTo[64:65, 0:128],
                    rhs=qTo[64:65, 0:512],
                    start=True, stop=False,
                    skip_group_check=True,
                )

                e_all = epool.tile([128, 2048 * nk], bf16, tag="e", name="e_all")
                for t in range(nk):
                    for j in range(2):  # head pairs (2j, 2j+1)
                        s_ps = spsum.tile([128, 1024], f32, tag="sc", name="s_ps")
                        for hh in range(2):
                            h = 2 * j + hh
                            nc.tensor.matmul(
                                s_ps[:, 512 * hh : 512 * (hh + 1)],
                                lhsT=kp[32 * h : 32 * h + 16, 128 * t : 128 * (t + 1)],
                                rhs=qp[32 * h : 32 * h + 16, :],
                                start=True, stop=True,
                                tile_position=(32 * h, 0),
                            )
                        nc.scalar.activation(
                            e_all[:, 2048 * t + 1024 * j : 2048 * t + 1024 * (j + 1)],
                            s_ps[:, :],
                            AF.Exp,
                            bias=mnegS[:, t : t + 1],
                            scale=0.25,
                        )
                    for h in range(H):
                        nc.tensor.matmul(
                            ctx_ps[32 * h : 32 * h + 17, :],
                            lhsT=vpo[:, 68 * t + 17 * h : 68 * t + 17 * (h + 1)],
                            rhs=e_all[:, 2048 * t + 512 * h : 2048 * t + 512 * (h + 1)],
                            start=False, stop=(t == nk - 1),
                            tile_position=(0, 32 * h),
                            skip_group_check=True,
                        )
                st["ctx_ps"] = ctx_ps
                st["e_all"] = e_all

            def ph2(b):
                """Denominators, reciprocals, scaled context -> attended."""
                st = state[b]
                ctx_ps = st["ctx_ps"]
                ctxs = workp.tile([128, 512], bf16, tag="ctxs", name="ctxs")
                nc.vector.tensor_scalar_add(ctxs[:, :], ctx_ps[:, :], -1.0)
                d_ps = ppsum.tile([4, 512], f32, tag="post", name="d_ps")
                nc.tensor.matmul(
                    d_ps[:, :], lhsT=sel_s[:, :], rhs=ctxs[:, :],
                    start=True, stop=True,
                )
                rstack = workp.tile([4, 512], f32, tag="rstack", name="rstack")
                nc.vector.reciprocal_approx_fast(out=rstack[:, :], in_=d_ps[:, :])
                rstackb = workp.tile([4, 512], bf16, tag="rstackb", name="rstackb")
                nc.vector.tensor_copy(rstackb[:, :], rstack[:, :])

                rbig = workp.tile([128, 2048], bf16, tag="rbig", name="rbig")
                for h in range(H):
                    r_ps = ppsum.tile([128, 512], f32, tag="post", name="r_ps")
                    nc.tensor.matmul(
                        r_ps[:, :],
                        lhsT=bsel_s[:, 128 * h : 128 * (h + 1)],
                        rhs=rstackb[:, :],
                        start=True, stop=True,
                    )
                    # r/4 for the head-mean path
                    nc.scalar.activation(
                        rbig[:, 512 * h : 512 * (h + 1)], r_ps[:, :],
                        AF.Copy, scale=0.25,
                    )
                st["rbig"] = rbig
                rs_ps = ppsum.tile([128, 512], f32, tag="post", name="rs_ps")
                nc.tensor.matmul(
                    rs_ps[:, :], lhsT=selb_s[:, :], rhs=rstackb[:, :],
                    start=True, stop=True,
                )
                rs_sb = workp.tile([128, 512], bf16, tag="rs_sb", name="rs_sb")
                nc.scalar.activation(rs_sb[:, :], rs_ps[:, :], AF.Copy, scale=0.25)
                st["rs_sb"] = rs_sb
                st["ctxs"] = ctxs

            def ph2b(b):
                """Scaled context -> attended (emitted after the deferred
                mean path so the DVE stream reaches it without stalling)."""
                st = state[b]
                ctxs, rs_sb = st["ctxs"], st["rs_sb"]
                catt = workp.tile([128, 512], bf16, tag="catt", name="catt")
                nc.vector.tensor_mul(catt[:, :], ctxs[:, :], rs_sb[:, :])

                atts = workp.tile([128, NC2 * 64], f32, tag="atts", name="atts")
                att_ps = ppsum.tile([128, NC2 * 64], f32, tag="post", name="att_ps")
                for c in range(NC2):
                    nc.tensor.matmul(
                        att_ps[:, 64 * c : 64 * (c + 1)],
                        lhsT=catt[:, 128 * c : 128 * (c + 1)],
                        rhs=wob_s[:, :],
                        start=True, stop=False,
                        skip_group_check=True,
                    )
                    nc.tensor.matmul(
                        att_ps[:, 64 * c : 64 * (c + 1)],
                        lhsT=qTbs[b][64:65, 0:128],
                        rhs=bob_s[64:65, :],
                        start=False, stop=True,
                        skip_group_check=True,
                    )
                nc.vector.tensor_copy(atts[:, :], att_ps[:, :])
                nc.sync.dma_start(
                    out=att[b].rearrange("(c p) d -> p c d", p=128),
                    in_=atts[:, :].rearrange("p (c d) -> p c d", d=64),
                )

            def ph3(b):
                """Head-mean max path (DVE heavy), deferred one batch."""
                st = state[b]
                e_all, rbig = st["e_all"], st["rbig"]
                mmax = workp.tile([128, 512], bf16, tag="mmax", name="mmax")
                for t in range(nk):
                    mprod = workp.tile([128, 2048], bf16, tag="mprod", name="mprod")
                    nc.vector.tensor_mul(
                        mprod[:, :], e_all[:, 2048 * t : 2048 * (t + 1)], rbig[:, :]
                    )
                    s01 = workp.tile([128, 1024], bf16, tag="s01", name="s01")
                    nc.vector.tensor_add(s01[:, :], mprod[:, 0:1024], mprod[:, 1024:2048])
                    if t == 0:
                        nc.vector.tensor_add(mmax[:, :], s01[:, 0:512], s01[:, 512:1024])
                    else:
                        macc = workp.tile([128, 512], bf16, tag="macc", name="macc")
                        nc.vector.tensor_add(macc[:, :], s01[:, 0:512], s01[:, 512:1024])
                        nc.vector.tensor_max(mmax[:, :], mmax[:, :], macc[:, :])

                # max over the 128 key partitions: PE transpose + free reduce
                mxcol = workp.tile([128, NC2], f32, tag="mxcol", name="mxcol")
                for c in range(NC2):
                    tp_ps = ppsum.tile([128, 128], bf16, tag="post", name="tp_ps")
                    nc.tensor.transpose(
                        tp_ps[:, :], mmax[:, 128 * c : 128 * (c + 1)], iden_s[:, :]
                    )
                    nc.vector.reduce_max(mxcol[:, c : c + 1], tp_ps[:, :], axis=AX.X)
                nc.sync.dma_start(
                    out=mx[b].rearrange("(c p) -> p c", p=128), in_=mxcol[:, :]
                )
                del state[b]

            # Software-pipelined schedule: the DVE-heavy mean path of batch b
            # is traced after batch b+1's compute phases so the DVE stream
            # doesn't stall the next batch's PE/ACT start.
            proj(0)
            for b in range(NB):
                ph1(b)
                if b + 1 < NB:
                    proj(b + 1)
                ph2(b)
                if b > 0:
                    ph3(b - 1)
                ph2b(b)
            ph3(NB - 1)

    nc.compile()
    return nc


def _prep(queries, keys, values, key_mask, wq, bq, wk, bk, wv, bv, wo, bo):
    """Host-side marshalling: shard over batch, transpose to [d, seq] layout,
    compact keys/values by the mask, build the fused weight/selector tensors."""
    q = np.ascontiguousarray(np.asarray(queries, np.float32))
    k = np.ascontiguousarray(np.asarray(keys, np.float32))
    v = np.ascontiguousarray(np.asarray(values, np.float32))
    mask = np.asarray(key_mask).astype(bool)

    counts = mask.sum(1)
    kmax = int(counts.max())
    nk = max(1, (kmax + 127) // 128)
    KP = 128 * nk

    kc = np.zeros((B, D_MODEL, KP), np.float32)
    vc = np.zeros((B, D_MODEL, KP), np.float32)
    mn = np.full((B, KP), NEG, np.float32)
    for b in range(B):
        idx = np.nonzero(mask[b])[0]
        c = len(idx)
        if c:
            kc[b, :, :c] = k[b, idx].T
            vc[b, :, :c] = v[b, idx].T
            mn[b, :c] = 0.0

    def _with_ones(x, nzero=0):  # [B, 64, S] -> [B, 65+nzero, S]
        ones = np.ones((B, 1, x.shape[2]), np.float32)
        parts = [x, ones]
        if nzero:
            parts.append(np.zeros((B, nzero, x.shape[2]), np.float32))
        return np.ascontiguousarray(np.concatenate(parts, axis=1))

    qT = _with_ones(q.transpose(0, 2, 1))
    kT = _with_ones(kc)
    vT = _with_ones(vc)
    mneg = np.ascontiguousarray(mn.reshape(B, nk, 128).transpose(0, 2, 1))

    wq = np.asarray(wq, np.float32); bq = np.asarray(bq, np.float32)
    wk = np.asarray(wk, np.float32); bk = np.asarray(bk, np.float32)
    wv = np.asarray(wv, np.float32); bv = np.asarray(bv, np.float32)
    wo = np.asarray(wo, np.float32); bo = np.asarray(bo, np.float32)

    wqb = np.concatenate([wq, bq[None]], 0).astype(np.float32)
    wkb = np.concatenate([wk, bk[None]], 0).astype(np.float32)
    wvo = np.zeros((65, 68), np.float32)
    for h in range(H):
        wvo[:64, 17 * h : 17 * h + 16] = wv[:, 16 * h : 16 * h + 16]
        wvo[64, 17 * h : 17 * h + 16] = bv[16 * h : 16 * h + 16]
        wvo[64, 17 * h + 16] = 1.0
    wob = np.zeros((128, 64), np.float32)
    for h in range(H):
        wob[32 * h : 32 * h + 16] = 4.0 * wo[16 * h : 16 * h + 16]
    selm = np.zeros((128, 4), np.float32)
    for h in range(H):
        selm[32 * h + 16, h] = 1.0
    bselm = np.zeros((4, 512), np.float32)
    for h in range(H):
        bselm[h, 128 * h : 128 * (h + 1)] = 1.0
    selbm = np.zeros((4, 128), np.float32)
    for h in range(H):
        selbm[h, 32 * h : 32 * h + 16] = 1.0

    import ml_dtypes
    bfd = ml_dtypes.bfloat16
    qT = qT.astype(bfd); kT = kT.astype(bfd); vT = vT.astype(bfd)
    bo65 = np.zeros((65, 64), np.float32)
    bo65[64] = bo
    cpk = np.zeros((128, 1096), np.float32)
    cpk[0:65, 0:64] = wqb
    cpk[0:65, 64:128] = wkb
    cpk[0:65, 128:196] = wvo
    cpk[:, 196:260] = wob
    cpk[0:65, 260:324] = bo65
    cpk[:, 324:328] = selm
    cpk[0:4, 328:840] = bselm
    cpk[0:4, 840:968] = selbm
    cpk[:, 968:1096] = np.eye(128)
    shared = dict(cpack=cpk.astype(bfd))
    in_maps = []
    for i in range(NCORES):
        s = slice(NB * i, NB * (i + 1))
        m = dict(qT=qT[s], kT=kT[s], vT=vT[s], mneg=mneg[s], **shared)
        in_maps.append(m)
    return nk, in_maps


def _ensure_ntff_hook():
    """The agent image's antenv lacks axon_hooks; synthesize it so
    run_bass_kernel_spmd(trace=True) can NTFF-profile via libaxon_pjrt.so."""
    import sys as _sys
    import types
    try:
        from antenv.axon_hooks import get_axon_ntff_profile_hook  # noqa: F401
        return
    except ImportError:
        pass
    mod = types.ModuleType("antenv.axon_hooks")
    _state = {"hook": None}
    mod.set_axon_ntff_profile_hook = lambda h: _state.__setitem__("hook", h)
    mod.get_axon_ntff_profile_hook = lambda: _state["hook"]
    _sys.modules["antenv.axon_hooks"] = mod
    import antenv
    antenv.axon_hooks = mod
    if "/root/.axon_site/trn_agent_boot" not in _sys.path:
        _sys.path.insert(0, "/root/.axon_site/trn_agent_boot")
    try:
        import trn_boot
        hook = trn_boot._ntff_profile_via_ctypes("/opt/axon/libaxon_pjrt.so")
        mod.set_axon_ntff_profile_hook(hook)
    except Exception as e:  # degrade: tracing skipped
        print(f"ntff hook setup failed: {e}", file=sys.stderr)


def _run(in_maps, nk, trace=False):
    global LAST_EXEC_NS, LAST_TRACE_DIR, LAST_RESULT
    from concourse.bass_utils import run_bass_kernel_spmd

    if trace:
        _ensure_ntff_hook()

    if nk not in _CACHE:
        _CACHE[nk] = _build(nk)
    nc = _CACHE[nk]
    res = run_bass_kernel_spmd(nc, in_maps, core_ids=list(range(NCORES)), trace=trace)
    LAST_RESULT = res
    if res.exec_time_ns is not None:
        LAST_EXEC_NS = res.exec_time_ns
    return res.results


def kernel(queries, keys, values, key_mask, wq, bq, wk, bk, wv, bv, wo, bo,
           _trace=False):
    nk, in_maps = _prep(queries, keys, values, key_mask,
                        wq, bq, wk, bk, wv, bv, wo, bo)
    results = _run(in_maps, nk, trace=_trace)
    attended = np.zeros((B, N, D_MODEL), np.float32)
    max_attn = np.zeros((B, N), np.float32)
    for i in range(NCORES):
        attended[NB * i : NB * (i + 1)] = np.asarray(results[i]["att"])
        max_attn[NB * i : NB * (i + 1)] = np.asarray(results[i]["mx"])
    # All-masked batches divide by a zero denominator on device; the
    # reference defines their outputs as bias / zeros. Patch on host.
    dead = ~np.asarray(key_mask).astype(bool).any(axis=1)
    if dead.any():
        attended[dead] = np.asarray(bo, np.float32)[None, None, :]
        max_attn[dead] = 0.0
    return attended, max_attn


# revision 37
# speedup vs baseline: 1.0055x; 1.0055x over previous
"""Trainium2 Bass kernel for nn_CrossAttentionModule (sparse cross-attention).

Strategy
--------
Data-parallel over batch: 32 batches -> 8 NeuronCores, 4 batches each. No
collectives. Per batch, everything is computed in "layout B" (keys on SBUF
partitions, query positions on the free dim):

  scores^T[k, n] = sum_d kp[d, k] * qp[d, n]     (PE, row-tiled: 4 heads packed)
  e^T = exp(0.25 * scores^T + maskneg[k])        (ScalarE, mask folded into the
                                                  per-partition bias; one pass)
  ctx^T[d, n]   = sum_k [vp | 1][k, d] * e^T     (PE, col-tiled 4 heads; the
                                                  ones column yields the softmax
                                                  denominator for free)
  r_h = 1 / denom_h                              (DVE reciprocal_approx_fast)
  attended^T    = (ctx^T * r) @ wo + bo          (PE)
  max_attn[n]   = max_k sum_h e_h^T[k,n] * r_h[n]/4
                                                 (DVE muls/adds with PE-broadcast
                                                  r tiles; final partition max via
                                                  PE transpose + free-dim reduce)

Sparsity: the key mask is known on the host, so keys/values are compacted
(gathered) on the host to the max unmasked count (padded to 128, pad columns
get -1e9 mask bias -> exp == 0). This roughly halves K for every engine.

All-masked batches (zero denominator) are patched on the host after the
run: the reference defines them as attended = bo, max_attn = 0.
"""

import sys

import numpy as np

for _p in ("/opt/trn_rl_repo",):
    if _p not in sys.path:
        sys.path.insert(0, _p)

B, N, D_MODEL = 32, 512, 64
K_FULL = 1024
H, DK = 4, 16
NCORES = 8
NB = B // NCORES  # batches per core
NEG = -1.0e9

_CACHE = {}
LAST_EXEC_NS = None
LAST_TRACE_DIR = None
LAST_RESULT = None


def _build(nk: int):
    """Build the SPMD Bass graph for one core processing NB batches with
    nk key-tiles of 128 (KP = 128*nk compacted keys)."""
    import concourse.bass as bass
    import concourse.bacc as bacc
    import concourse.mybir as mybir
    from concourse.tile import TileContext

    f32 = mybir.dt.float32
    bf16 = mybir.dt.bfloat16
    AF = mybir.ActivationFunctionType
    AX = mybir.AxisListType

    KP = 128 * nk
    NC2 = N // 128  # n-chunks of 128 (=4)

    nc = bacc.Bacc()
    qT = nc.declare_dram_parameter("qT", [NB, 65, N], bf16, isOutput=False)
    kT = nc.declare_dram_parameter("kT", [NB, 65, KP], bf16, isOutput=False)
    vT = nc.declare_dram_parameter("vT", [NB, 65, KP], bf16, isOutput=False)
    mneg = nc.declare_dram_parameter("mneg", [NB, 128, nk], f32, isOutput=False)
    cpack = nc.declare_dram_parameter("cpack", [128, 1096], bf16, isOutput=False)
    att = nc.declare_dram_parameter("att", [NB, N, 64], f32, isOutput=True)
    mx = nc.declare_dram_parameter("mx", [NB, N], f32, isOutput=True)

    with TileContext(nc) as tc:
        with (
            tc.tile_pool(name="consts", bufs=1) as cpool,
            tc.tile_pool(name="io", bufs=2) as iop,
            tc.tile_pool(name="proj", bufs=2) as projp,
            tc.tile_pool(name="epool", bufs=3) as epool,
            tc.tile_pool(name="work", bufs=3) as workp,
            tc.tile_pool(name="spsum", bufs=2, space="PSUM") as spsum,
            tc.tile_pool(name="cpsum", bufs=2, space="PSUM") as cpsum,
            tc.tile_pool(name="ppsum", bufs=2, space="PSUM") as ppsum,
        ):
            cp = cpool.tile_from(cpack[:, :])
            # column layout: wqb(64) wkb(64) wvo(68) wob(64) bob(64)
            #                sel(4) bsel(512) selb(128) iden(128)
            wqb_s = cp[0:65, 0:64]
            wkb_s = cp[0:65, 64:128]
            wvo_s = cp[0:65, 128:196]
            wob_s = cp[:, 196:260]
            bob_s = cp[0:65, 260:324]
            sel_s = cp[:, 324:328]
            bsel_s = cp[0:4, 328:840]
            selb_s = cp[0:4, 840:968]
            iden_s = cp[:, 968:1096]

            # ---- load all batches once ----------------------------------
            qTbs, kTbs, vTbs, mnegbs = [], [], [], []
            for b in range(NB):
                eng = nc.sync if b == 0 else nc.gpsimd
                mnegb = iop.tile([128, nk], f32, tag="mnegb", name="mnegb", bufs=NB)
                eng.dma_start(out=mnegb[:, :], in_=mneg[b])
                qTb = iop.tile([65, N], bf16, tag="qTb", name="qTb", bufs=NB)
                eng.dma_start(out=qTb[:, :], in_=qT[b])
                kTb = iop.tile([65, KP], bf16, tag="kTb", name="kTb", bufs=NB)
                eng.dma_start(out=kTb[:, :], in_=kT[b])
                vTb = iop.tile([65, KP], bf16, tag="vTb", name="vTb", bufs=NB)
                eng.dma_start(out=vTb[:, :], in_=vT[b])
                qTbs.append(qTb); kTbs.append(kTb); vTbs.append(vTb); mnegbs.append(mnegb)


            state = {}

            def proj(b):
                """Q/K/V projections (emitted mid-k-loop of the previous
                batch, when scores-psum slots are free)."""
                qTo, kTo, vTo = qTbs[b], kTbs[b], vTbs[b]
                qp_ps = spsum.tile([128, 512], f32, tag="sc", name="qp_ps")
                for h in range(H):
                    nc.tensor.matmul(
                        qp_ps[32 * h : 32 * h + 16, :],
                        lhsT=wqb_s[:, 16 * h : 16 * h + 16],
                        rhs=qTo[:, :],
                        start=True, stop=True,
                        tile_position=(0, 32 * h),
                    )
                qp = projp.tile([128, N], bf16, tag="qp", name="qp")
                nc.scalar.copy(qp[:, :], qp_ps[:, :])

                kp = projp.tile([128, KP], bf16, tag="kp", name="kp")
                kp_ps = spsum.tile([128, 1024], f32, tag="sc", name="kp_ps")
                for c0 in range(0, KP, 512):
                    w = min(512, KP - c0)
                    for h in range(H):
                        nc.tensor.matmul(
                            kp_ps[32 * h : 32 * h + 16, c0 : c0 + w],
                            lhsT=wkb_s[:, 16 * h : 16 * h + 16],
                            rhs=kTo[:, c0 : c0 + w],
                            start=True, stop=True,
                            tile_position=(0, 32 * h),
                        )
                nc.scalar.copy(kp[:, :], kp_ps[:, :KP])

                vpo = projp.tile([128, 68 * nk], bf16, tag="vpo", name="vpo")
                vp_ps = spsum.tile([128, 68 * nk], f32, tag="sc", name="vp_ps")
                for t in range(nk):
                    nc.tensor.matmul(
                        vp_ps[:, 68 * t : 68 * (t + 1)],
                        lhsT=vTo[:, 128 * t : 128 * (t + 1)],
                        rhs=wvo_s[:, :],
                        start=True, stop=True,
                    )
                nc.scalar.copy(vpo[:, :], vp_ps[:, :])
                state[b] = dict(qp=qp, kp=kp, vpo=vpo)

            def ph1(b):
                """Scores/exp/context k-loop (PE+ACT heavy)."""
                qTo, mnegS = qTbs[b], mnegbs[b]
                st = state[b]
                qp, kp, vpo = st["qp"], st["kp"], st["vpo"]
                # ctx^T accumulator: head h rows 32h..32h+15, denom row 32h+16.
                ctx_ps = cpsum.tile([128, 512], f32, tag="cq", name="ctx_ps")

                # Ones matmul: writes 1.0 everywhere, clears + sets the whole
                # bank's has_written bits so the interleaved head groups can
                # accumulate with start=False in any order; the 1.0 is
                # subtracted in the ctxs evacuation (bias=-1).
                nc.tensor.matmul(
                    ctx_ps[:, :],
                    lhsT=qTo[64:65, 0:128],
                    rhs=qTo[64:65, 0:512],
                    start=True, stop=False,
                    skip_group_check=True,
                )

                e_all = epool.tile([128, 2048 * nk], bf16, tag="e", name="e_all")
                for t in range(nk):
                    for j in range(2):  # head pairs (2j, 2j+1)
                        s_ps = spsum.tile([128, 1024], f32, tag="sc", name="s_ps")
                        for hh in range(2):
                            h = 2 * j + hh
                            nc.tensor.matmul(
                                s_ps[:, 512 * hh : 512 * (hh + 1)],
                                lhsT=kp[32 * h : 32 * h + 16, 128 * t : 128 * (t + 1)],
                                rhs=qp[32 * h : 32 * h + 16, :],
                                start=True, stop=True,
                                tile_position=(32 * h, 0),
                            )
                        nc.scalar.activation(
                            e_all[:, 2048 * t + 1024 * j : 2048 * t + 1024 * (j + 1)],
                            s_ps[:, :],
                            AF.Exp,
                            bias=mnegS[:, t : t + 1],
                            scale=0.25,
                        )
                    for h in range(H):
                        nc.tensor.matmul(
                            ctx_ps[32 * h : 32 * h + 17, :],
                            lhsT=vpo[:, 68 * t + 17 * h : 68 * t + 17 * (h + 1)],
                            rhs=e_all[:, 2048 * t + 512 * h : 2048 * t + 512 * (h + 1)],
                            start=False, stop=(t == nk - 1),
                            tile_position=(0, 32 * h),
                            skip_group_check=True,
                        )
                st["ctx_ps"] = ctx_ps
                st["e_all"] = e_all

            def ph2(b):
                """Denominators, reciprocals, scaled context -> attended."""
                st = state[b]
                ctx_ps = st["ctx_ps"]
                ctxs = workp.tile([128, 512], bf16, tag="ctxs", name="ctxs")
                nc.vector.tensor_scalar_add(ctxs[:, :], ctx_ps[:, :], -1.0)
                d_ps = ppsum.tile([4, 512], f32, tag="post", name="d_ps")
                nc.tensor.matmul(
                    d_ps[:, :], lhsT=sel_s[:, :], rhs=ctxs[:, :],
                    start=True, stop=True,
                )
                rstack = workp.tile([4, 512], f32, tag="rstack", name="rstack")
                nc.vector.reciprocal_approx_fast(out=rstack[:, :], in_=d_ps[:, :])
                rstackb = workp.tile([4, 512], bf16, tag="rstackb", name="rstackb")
                nc.vector.tensor_copy(rstackb[:, :], rstack[:, :])

                rbig = workp.tile([128, 2048], bf16, tag="rbig", name="rbig")
                for h in range(H):
                    r_ps = ppsum.tile([128, 512], f32, tag="post", name="r_ps")
                    nc.tensor.matmul(
                        r_ps[:, :],
                        lhsT=bsel_s[:, 128 * h : 128 * (h + 1)],
                        rhs=rstackb[:, :],
                        start=True, stop=True,
                    )
                    # r/4 for the head-mean path
                    nc.scalar.activation(
                        rbig[:, 512 * h : 512 * (h + 1)], r_ps[:, :],
                        AF.Copy, scale=0.25,
                    )
                st["rbig"] = rbig
                rs_ps = ppsum.tile([128, 512], f32, tag="post", name="rs_ps")
                nc.tensor.matmul(
                    rs_ps[:, :], lhsT=selb_s[:, :], rhs=rstackb[:, :],
                    start=True, stop=True,
                )
                rs_sb = workp.tile([128, 512], bf16, tag="rs_sb", name="rs_sb")
                nc.scalar.activation(rs_sb[:, :], rs_ps[:, :], AF.Copy, scale=0.25)
                st["rs_sb"] = rs_sb
                st["ctxs"] = ctxs

            def ph2b(b):
                """Scaled context -> attended (emitted after the deferred
                mean path so the DVE stream reaches it without stalling)."""
                st = state[b]
                ctxs, rs_sb = st["ctxs"], st["rs_sb"]
                catt = workp.tile([128, 512], bf16, tag="catt", name="catt")
                nc.vector.tensor_mul(catt[:, :], ctxs[:, :], rs_sb[:, :])

                atts = workp.tile([128, NC2 * 64], f32, tag="atts", name="atts")
                att_ps = ppsum.tile([128, NC2 * 64], f32, tag="post", name="att_ps")
                for c in range(NC2):
                    nc.tensor.matmul(
                        att_ps[:, 64 * c : 64 * (c + 1)],
                        lhsT=catt[:, 128 * c : 128 * (c + 1)],
                        rhs=wob_s[:, :],
                        start=True, stop=False,
                        skip_group_check=True,
                    )
                    nc.tensor.matmul(
                        att_ps[:, 64 * c : 64 * (c + 1)],
                        lhsT=qTbs[b][64:65, 0:128],
                        rhs=bob_s[64:65, :],
                        start=False, stop=True,
                        skip_group_check=True,
                    )
                nc.vector.tensor_copy(atts[:, :], att_ps[:, :])
                nc.sync.dma_start(
                    out=att[b].rearrange("(c p) d -> p c d", p=128),
                    in_=atts[:, :].rearrange("p (c d) -> p c d", d=64),
                )

            def ph3(b):
                """Head-mean max path (DVE heavy), deferred one batch."""
                st = state[b]
                e_all, rbig = st["e_all"], st["rbig"]
                mmax = workp.tile([128, 512], bf16, tag="mmax", name="mmax")
                for t in range(nk):
                    mprod = workp.tile([128, 2048], bf16, tag="mprod", name="mprod")
                    nc.vector.tensor_mul(
                        mprod[:, :], e_all[:, 2048 * t : 2048 * (t + 1)], rbig[:, :]
                    )
                    s01 = workp.tile([128, 1024], bf16, tag="s01", name="s01")
                    nc.vector.tensor_add(s01[:, :], mprod[:, 0:1024], mprod[:, 1024:2048])
                    if t == 0:
                        nc.vector.tensor_add(mmax[:, :], s01[:, 0:512], s01[:, 512:1024])
                    else:
                        macc = workp.tile([128, 512], bf16, tag="macc", name="macc")
                        nc.vector.tensor_add(macc[:, :], s01[:, 0:512], s01[:, 512:1024])
                        nc.vector.tensor_max(mmax[:, :], mmax[:, :], macc[:, :])

                # max over the 128 key partitions: PE transpose + free reduce
                mxcol = workp.tile([128, NC2], f32, tag="mxcol", name="mxcol")
                for c in range(NC2):
                    tp_ps = ppsum.tile([128, 128], bf16, tag="post", name="tp_ps")
                    nc.tensor.transpose(
                        tp_ps[:, :], mmax[:, 128 * c : 128 * (c + 1)], iden_s[:, :]
                    )
                    nc.vector.reduce_max(mxcol[:, c : c + 1], tp_ps[:, :], axis=AX.X)
                nc.sync.dma_start(
                    out=mx[b].rearrange("(c p) -> p c", p=128), in_=mxcol[:, :]
                )
                del state[b]

            # Software-pipelined schedule: the DVE-heavy mean path of batch b
            # is traced after batch b+1's compute phases so the DVE stream
            # doesn't stall the next batch's PE/ACT start.
            proj(0)
            for b in range(NB):
                ph1(b)
                if b + 1 < NB:
                    proj(b + 1)
                ph2(b)
                if b > 0:
                    ph3(b - 1)
                ph2b(b)
            ph3(NB - 1)

    nc.compile()
    return nc


def _prep(queries, keys, values, key_mask, wq, bq, wk, bk, wv, bv, wo, bo):
    """Host-side marshalling: shard over batch, transpose to [d, seq] layout,
    compact keys/values by the mask, build the fused weight/selector tensors."""
    q = np.ascontiguousarray(np.asarray(queries, np.float32))
    k = np.ascontiguousarray(np.asarray(keys, np.float32))
    v = np.ascontiguousarray(np.asarray(values, np.float32))
    mask = np.asarray(key_mask).astype(bool)

    counts = mask.sum(1)
    kmax = int(counts.max())
    nk = max(1, (kmax + 127) // 128)
    KP = 128 * nk

    kc = np.zeros((B, D_MODEL, KP), np.float32)
    vc = np.zeros((B, D_MODEL, KP), np.float32)
    mn = np.full((B, KP), NEG, np.float32)
    for b in range(B):
        idx = np.nonzero(mask[b])[0]
        c = len(idx)
        if c:
            kc[b, :, :c] = k[b, idx].T
            vc[b, :, :c] = v[b, idx].T
            mn[b, :c] = 0.0

    def _with_ones(x, nzero=0):  # [B, 64, S] -> [B, 65+nzero, S]
        ones = np.ones((B, 1, x.shape[2]), np.float32)
        parts = [x, ones]
        if nzero:
            parts.append(np.zeros((B, nzero, x.shape[2]), np.float32))
        return np.ascontiguousarray(np.concatenate(parts, axis=1))

    qT = _with_ones(q.transpose(0, 2, 1))
    kT = _with_ones(kc)
    vT = _with_ones(vc)
    mneg = np.ascontiguousarray(mn.reshape(B, nk, 128).transpose(0, 2, 1))

    wq = np.asarray(wq, np.float32); bq = np.asarray(bq, np.float32)
    wk = np.asarray(wk, np.float32); bk = np.asarray(bk, np.float32)
    wv = np.asarray(wv, np.float32); bv = np.asarray(bv, np.float32)
    wo = np.asarray(wo, np.float32); bo = np.asarray(bo, np.float32)

    wqb = np.concatenate([wq, bq[None]], 0).astype(np.float32)
    wkb = np.concatenate([wk, bk[None]], 0).astype(np.float32)
    wvo = np.zeros((65, 68), np.float32)
    for h in range(H):
        wvo[:64, 17 * h : 17 * h + 16] = wv[:, 16 * h : 16 * h + 16]
        wvo[64, 17 * h : 17 * h + 16] = bv[16 * h : 16 * h + 16]
        wvo[64, 17 * h + 16] = 1.0
    wob = np.zeros((128, 64), np.float32)
    for h in range(H):
        wob[32 * h : 32 * h + 16] = 4.0 * wo[16 * h : 16 * h + 16]
    selm = np.zeros((128, 4), np.float32)
    for h in range(H):
        selm[32 * h + 16, h] = 1.0
    bselm = np.zeros((4, 512), np.float32)
    for h in range(H):
        bselm[h, 128 * h : 128 * (h + 1)] = 1.0
    selbm = np.zeros((4, 128), np.float32)
    for h in range(H):
        selbm[h, 32 * h : 32 * h + 16] = 1.0

    import ml_dtypes
    bfd = ml_dtypes.bfloat16
    qT = qT.astype(bfd); kT = kT.astype(bfd); vT = vT.astype(bfd)
    bo65 = np.zeros((65, 64), np.float32)
    bo65[64] = bo
    cpk = np.zeros((128, 1096), np.float32)
    cpk[0:65, 0:64] = wqb
    cpk[0:65, 64:128] = wkb
    cpk[0:65, 128:196] = wvo
    cpk[:, 196:260] = wob
    cpk[0:65, 260:324] = bo65
    cpk[:, 324:328] = selm
    cpk[0:4, 328:840] = bselm
    cpk[0:4, 840:968] = selbm
    cpk[:, 968:1096] = np.eye(128)
    shared = dict(cpack=cpk.astype(bfd))
    in_maps = []
    for i in range(NCORES):
        s = slice(NB * i, NB * (i + 1))
        m = dict(qT=qT[s], kT=kT[s], vT=vT[s], mneg=mneg[s], **shared)
        in_maps.append(m)
    return nk, in_maps


def _ensure_ntff_hook():
    """The agent image's antenv lacks axon_hooks; synthesize it so
    run_bass_kernel_spmd(trace=True) can NTFF-profile via libaxon_pjrt.so."""
    import sys as _sys
    import types
    try:
        from antenv.axon_hooks import get_axon_ntff_profile_hook  # noqa: F401
        return
    except ImportError:
        pass
    mod = types.ModuleType("antenv.axon_hooks")
    _state = {"hook": None}
    mod.set_axon_ntff_profile_hook = lambda h: _state.__setitem__("hook", h)
    mod.get_axon_ntff_profile_hook = lambda: _state["hook"]
    _sys.modules["antenv.axon_hooks"] = mod
    import antenv
    antenv.axon_hooks = mod
    if "/root/.axon_site/trn_agent_boot" not in _sys.path:
        _sys.path.insert(0, "/root/.axon_site/trn_agent_boot")
    try:
        import trn_boot
        hook = trn_boot._ntff_profile_via_ctypes("/opt/axon/libaxon_pjrt.so")
        mod.set_axon_ntff_profile_hook(hook)
    except Exception as e:  # degrade: tracing skipped
        print(f"ntff hook setup failed: {e}", file=sys.stderr)


def _run(in_maps, nk, trace=False):
    global LAST_EXEC_NS, LAST_TRACE_DIR, LAST_RESULT
    from concourse.bass_utils import run_bass_kernel_spmd

    if trace:
        _ensure_ntff_hook()

    if nk not in _CACHE:
        _CACHE[nk] = _build(nk)
    nc = _CACHE[nk]
    res = run_bass_kernel_spmd(nc, in_maps, core_ids=list(range(NCORES)), trace=trace)
    LAST_RESULT = res
    if res.exec_time_ns is not None:
        LAST_EXEC_NS = res.exec_time_ns
    return res.results


def kernel(queries, keys, values, key_mask, wq, bq, wk, bk, wv, bv, wo, bo,
           _trace=False):
    nk, in_maps = _prep(queries, keys, values, key_mask,
                        wq, bq, wk, bk, wv, bv, wo, bo)
    results = _run(in_maps, nk, trace=_trace)
    attended = np.zeros((B, N, D_MODEL), np.float32)
    max_attn = np.zeros((B, N), np.float32)
    for i in range(NCORES):
        attended[NB * i : NB * (i + 1)] = np.asarray(results[i]["att"])
        max_attn[NB * i : NB * (i + 1)] = np.asarray(results[i]["mx"])
    # All-masked batches divide by a zero denominator on device; the
    # reference defines their outputs as bias / zeros. Patch on host.
    dead = ~np.asarray(key_mask).astype(bool).any(axis=1)
    if dead.any():
        attended[dead] = np.asarray(bo, np.float32)[None, None, :]
        max_attn[dead] = 0.0
    return attended, max_attn


# revision 38
# speedup vs baseline: 1.0226x; 1.0171x over previous
"""Trainium2 Bass kernel for nn_CrossAttentionModule (sparse cross-attention).

Strategy
--------
Data-parallel over batch: 32 batches -> 8 NeuronCores, 4 batches each. No
collectives. Per batch, everything is computed in "layout B" (keys on SBUF
partitions, query positions on the free dim):

  scores^T[k, n] = sum_d kp[d, k] * qp[d, n]     (PE, row-tiled: 4 heads packed)
  e^T = exp(0.25 * scores^T + maskneg[k])        (ScalarE, mask folded into the
                                                  per-partition bias; one pass)
  ctx^T[d, n]   = sum_k [vp | 1][k, d] * e^T     (PE, col-tiled 4 heads; the
                                                  ones column yields the softmax
                                                  denominator for free)
  r_h = 1 / denom_h                              (DVE reciprocal_approx_fast)
  attended^T    = (ctx^T * r) @ wo + bo          (PE)
  max_attn[n]   = max_k sum_h e_h^T[k,n] * r_h[n]/4
                                                 (DVE muls/adds with PE-broadcast
                                                  r tiles; final partition max via
                                                  PE transpose + free-dim reduce)

Sparsity: the key mask is known on the host, so keys/values are compacted
(gathered) on the host to the max unmasked count (padded to 128, pad columns
get -1e9 mask bias -> exp == 0). This roughly halves K for every engine.

All-masked batches (zero denominator) are patched on the host after the
run: the reference defines them as attended = bo, max_attn = 0.
"""

import sys

import numpy as np

for _p in ("/opt/trn_rl_repo",):
    if _p not in sys.path:
        sys.path.insert(0, _p)

B, N, D_MODEL = 32, 512, 64
K_FULL = 1024
H, DK = 4, 16
NCORES = 8
NB = B // NCORES  # batches per core
NEG = -1.0e9

_CACHE = {}
LAST_EXEC_NS = None
LAST_TRACE_DIR = None
LAST_RESULT = None


def _build(nk: int):
    """Build the SPMD Bass graph for one core processing NB batches with
    nk key-tiles of 128 (KP = 128*nk compacted keys)."""
    import concourse.bass as bass
    import concourse.bacc as bacc
    import concourse.mybir as mybir
    from concourse.tile import TileContext

    f32 = mybir.dt.float32
    bf16 = mybir.dt.bfloat16
    AF = mybir.ActivationFunctionType
    AX = mybir.AxisListType

    KP = 128 * nk
    NC2 = N // 128  # n-chunks of 128 (=4)

    nc = bacc.Bacc()
    qT = nc.declare_dram_parameter("qT", [NB, 65, N], bf16, isOutput=False)
    kT = nc.declare_dram_parameter("kT", [NB, 65, KP], bf16, isOutput=False)
    vT = nc.declare_dram_parameter("vT", [NB, 65, KP], bf16, isOutput=False)
    mneg = nc.declare_dram_parameter("mneg", [NB, 128, nk], f32, isOutput=False)
    cpack = nc.declare_dram_parameter("cpack", [128, 1096], bf16, isOutput=False)
    att = nc.declare_dram_parameter("att", [NB, N, 64], f32, isOutput=True)
    mx = nc.declare_dram_parameter("mx", [NB, N], f32, isOutput=True)

    with TileContext(nc) as tc:
        with (
            tc.tile_pool(name="consts", bufs=1) as cpool,
            tc.tile_pool(name="io", bufs=2) as iop,
            tc.tile_pool(name="proj", bufs=3) as projp,
            tc.tile_pool(name="epool", bufs=3) as epool,
            tc.tile_pool(name="work", bufs=3) as workp,
            tc.tile_pool(name="spsum", bufs=2, space="PSUM") as spsum,
            tc.tile_pool(name="cpsum", bufs=2, space="PSUM") as cpsum,
            tc.tile_pool(name="ppsum", bufs=2, space="PSUM") as ppsum,
        ):
            cp = cpool.tile_from(cpack[:, :])
            # column layout: wqb(64) wkb(64) wvo(68) wob(64) bob(64)
            #                sel(4) bsel(512) selb(128) iden(128)
            wqb_s = cp[0:65, 0:64]
            wkb_s = cp[0:65, 64:128]
            wvo_s = cp[0:65, 128:196]
            wob_s = cp[:, 196:260]
            bob_s = cp[0:65, 260:324]
            sel_s = cp[:, 324:328]
            bsel_s = cp[0:4, 328:840]
            selb_s = cp[0:4, 840:968]
            iden_s = cp[:, 968:1096]

            # ---- load all batches once ----------------------------------
            qTbs, kTbs, vTbs, mnegbs = [], [], [], []
            for b in range(NB):
                eng = nc.sync if b == 0 else nc.gpsimd
                mnegb = iop.tile([128, nk], f32, tag="mnegb", name="mnegb", bufs=NB)
                eng.dma_start(out=mnegb[:, :], in_=mneg[b])
                qTb = iop.tile([65, N], bf16, tag="qTb", name="qTb", bufs=NB)
                eng.dma_start(out=qTb[:, :], in_=qT[b])
                kTb = iop.tile([65, KP], bf16, tag="kTb", name="kTb", bufs=NB)
                eng.dma_start(out=kTb[:, :], in_=kT[b])
                vTb = iop.tile([65, KP], bf16, tag="vTb", name="vTb", bufs=NB)
                eng.dma_start(out=vTb[:, :], in_=vT[b])
                qTbs.append(qTb); kTbs.append(kTb); vTbs.append(vTb); mnegbs.append(mnegb)


            state = {}

            def proj(b):
                """Q/K/V projections (emitted mid-k-loop of the previous
                batch, when scores-psum slots are free)."""
                qTo, kTo, vTo = qTbs[b], kTbs[b], vTbs[b]
                qp_ps = spsum.tile([128, 512], f32, tag="sc", name="qp_ps")
                for h in range(H):
                    nc.tensor.matmul(
                        qp_ps[32 * h : 32 * h + 16, :],
                        lhsT=wqb_s[:, 16 * h : 16 * h + 16],
                        rhs=qTo[:, :],
                        start=True, stop=True,
                        tile_position=(0, 32 * h),
                    )
                qp = projp.tile([128, N], bf16, tag="qp", name="qp")
                nc.scalar.copy(qp[:, :], qp_ps[:, :])

                kp = projp.tile([128, KP], bf16, tag="kp", name="kp")
                kp_ps = spsum.tile([128, 1024], f32, tag="sc", name="kp_ps")
                for c0 in range(0, KP, 512):
                    w = min(512, KP - c0)
                    for h in range(H):
                        nc.tensor.matmul(
                            kp_ps[32 * h : 32 * h + 16, c0 : c0 + w],
                            lhsT=wkb_s[:, 16 * h : 16 * h + 16],
                            rhs=kTo[:, c0 : c0 + w],
                            start=True, stop=True,
                            tile_position=(0, 32 * h),
                        )
                nc.scalar.copy(kp[:, :], kp_ps[:, :KP])

                vpo = projp.tile([128, 68 * nk], bf16, tag="vpo", name="vpo")
                vp_ps = spsum.tile([128, 68 * nk], f32, tag="sc", name="vp_ps")
                for t in range(nk):
                    nc.tensor.matmul(
                        vp_ps[:, 68 * t : 68 * (t + 1)],
                        lhsT=vTo[:, 128 * t : 128 * (t + 1)],
                        rhs=wvo_s[:, :],
                        start=True, stop=True,
                    )
                nc.scalar.copy(vpo[:, :], vp_ps[:, :])
                state[b] = dict(qp=qp, kp=kp, vpo=vpo)

            def ph1(b):
                """Scores/exp/context k-loop (PE+ACT heavy)."""
                qTo, mnegS = qTbs[b], mnegbs[b]
                st = state[b]
                qp, kp, vpo = st["qp"], st["kp"], st["vpo"]
                # ctx^T accumulator: head h rows 32h..32h+15, denom row 32h+16.
                ctx_ps = cpsum.tile([128, 512], f32, tag="cq", name="ctx_ps")

                # Ones matmul: writes 1.0 everywhere, clears + sets the whole
                # bank's has_written bits so the interleaved head groups can
                # accumulate with start=False in any order; the 1.0 is
                # subtracted in the ctxs evacuation (bias=-1).
                nc.tensor.matmul(
                    ctx_ps[:, :],
                    lhsT=qTo[64:65, 0:128],
                    rhs=qTo[64:65, 0:512],
                    start=True, stop=False,
                    skip_group_check=True,
                )

                e_all = epool.tile([128, 2048 * nk], bf16, tag="e", name="e_all")
                for t in range(nk):
                    for j in range(2):  # head pairs (2j, 2j+1)
                        s_ps = spsum.tile([128, 1024], f32, tag="sc", name="s_ps")
                        for hh in range(2):
                            h = 2 * j + hh
                            nc.tensor.matmul(
                                s_ps[:, 512 * hh : 512 * (hh + 1)],
                                lhsT=kp[32 * h : 32 * h + 16, 128 * t : 128 * (t + 1)],
                                rhs=qp[32 * h : 32 * h + 16, :],
                                start=True, stop=True,
                                tile_position=(32 * h, 0),
                            )
                        nc.scalar.activation(
                            e_all[:, 2048 * t + 1024 * j : 2048 * t + 1024 * (j + 1)],
                            s_ps[:, :],
                            AF.Exp,
                            bias=mnegS[:, t : t + 1],
                            scale=0.25,
                        )
                    for h in range(H):
                        nc.tensor.matmul(
                            ctx_ps[32 * h : 32 * h + 17, :],
                            lhsT=vpo[:, 68 * t + 17 * h : 68 * t + 17 * (h + 1)],
                            rhs=e_all[:, 2048 * t + 512 * h : 2048 * t + 512 * (h + 1)],
                            start=False, stop=(t == nk - 1),
                            tile_position=(0, 32 * h),
                            skip_group_check=True,
                        )
                st["ctx_ps"] = ctx_ps
                st["e_all"] = e_all

            def ph2(b):
                """Denominators, reciprocals, scaled context -> attended."""
                st = state[b]
                ctx_ps = st["ctx_ps"]
                ctxs = workp.tile([128, 512], bf16, tag="ctxs", name="ctxs")
                nc.vector.tensor_scalar_add(ctxs[:, :], ctx_ps[:, :], -1.0)
                d_ps = ppsum.tile([4, 512], f32, tag="post", name="d_ps")
                nc.tensor.matmul(
                    d_ps[:, :], lhsT=sel_s[:, :], rhs=ctxs[:, :],
                    start=True, stop=True,
                )
                rstack = workp.tile([4, 512], f32, tag="rstack", name="rstack")
                nc.vector.reciprocal_approx_fast(out=rstack[:, :], in_=d_ps[:, :])
                rstackb = workp.tile([4, 512], bf16, tag="rstackb", name="rstackb")
                nc.vector.tensor_copy(rstackb[:, :], rstack[:, :])

                rbig = workp.tile([128, 2048], bf16, tag="rbig", name="rbig")
                for h in range(H):
                    r_ps = ppsum.tile([128, 512], f32, tag="post", name="r_ps")
                    nc.tensor.matmul(
                        r_ps[:, :],
                        lhsT=bsel_s[:, 128 * h : 128 * (h + 1)],
                        rhs=rstackb[:, :],
                        start=True, stop=True,
                    )
                    # r/4 for the head-mean path
                    nc.scalar.activation(
                        rbig[:, 512 * h : 512 * (h + 1)], r_ps[:, :],
                        AF.Copy, scale=0.25,
                    )
                st["rbig"] = rbig
                rs_ps = ppsum.tile([128, 512], f32, tag="post", name="rs_ps")
                nc.tensor.matmul(
                    rs_ps[:, :], lhsT=selb_s[:, :], rhs=rstackb[:, :],
                    start=True, stop=True,
                )
                rs_sb = workp.tile([128, 512], bf16, tag="rs_sb", name="rs_sb")
                nc.scalar.activation(rs_sb[:, :], rs_ps[:, :], AF.Copy, scale=0.25)
                st["rs_sb"] = rs_sb
                st["ctxs"] = ctxs

            def ph2b(b):
                """Scaled context -> attended (emitted after the deferred
                mean path so the DVE stream reaches it without stalling)."""
                st = state[b]
                ctxs, rs_sb = st["ctxs"], st["rs_sb"]
                catt = workp.tile([128, 512], bf16, tag="catt", name="catt")
                nc.vector.tensor_mul(catt[:, :], ctxs[:, :], rs_sb[:, :])

                atts = workp.tile([128, NC2 * 64], f32, tag="atts", name="atts")
                att_ps = ppsum.tile([128, NC2 * 64], f32, tag="post", name="att_ps")
                for c in range(NC2):
                    nc.tensor.matmul(
                        att_ps[:, 64 * c : 64 * (c + 1)],
                        lhsT=catt[:, 128 * c : 128 * (c + 1)],
                        rhs=wob_s[:, :],
                        start=True, stop=False,
                        skip_group_check=True,
                    )
                    nc.tensor.matmul(
                        att_ps[:, 64 * c : 64 * (c + 1)],
                        lhsT=qTbs[b][64:65, 0:128],
                        rhs=bob_s[64:65, :],
                        start=False, stop=True,
                        skip_group_check=True,
                    )
                nc.vector.tensor_copy(atts[:, :], att_ps[:, :])
                nc.sync.dma_start(
                    out=att[b].rearrange("(c p) d -> p c d", p=128),
                    in_=atts[:, :].rearrange("p (c d) -> p c d", d=64),
                )

            def ph3(b):
                """Head-mean max path (DVE heavy), deferred one batch."""
                st = state[b]
                e_all, rbig = st["e_all"], st["rbig"]
                mmax = workp.tile([128, 512], bf16, tag="mmax", name="mmax")
                for t in range(nk):
                    mprod = workp.tile([128, 2048], bf16, tag="mprod", name="mprod")
                    nc.vector.tensor_mul(
                        mprod[:, :], e_all[:, 2048 * t : 2048 * (t + 1)], rbig[:, :]
                    )
                    s01 = workp.tile([128, 1024], bf16, tag="s01", name="s01")
                    nc.vector.tensor_add(s01[:, :], mprod[:, 0:1024], mprod[:, 1024:2048])
                    if t == 0:
                        nc.vector.tensor_add(mmax[:, :], s01[:, 0:512], s01[:, 512:1024])
                    else:
                        macc = workp.tile([128, 512], bf16, tag="macc", name="macc")
                        nc.vector.tensor_add(macc[:, :], s01[:, 0:512], s01[:, 512:1024])
                        nc.vector.tensor_max(mmax[:, :], mmax[:, :], macc[:, :])

                # max over the 128 key partitions: PE transpose + free reduce
                mxcol = workp.tile([128, NC2], f32, tag="mxcol", name="mxcol")
                tp_ps = ppsum.tile([128, NC2 * 128], bf16, tag="post", name="tp_ps")
                for c in range(NC2):
                    nc.tensor.transpose(
                        tp_ps[:, 128 * c : 128 * (c + 1)],
                        mmax[:, 128 * c : 128 * (c + 1)], iden_s[:, :]
                    )
                nc.vector.reduce_max(
                    mxcol[:, :],
                    tp_ps[:, :].rearrange("p (c k) -> p c k", k=128),
                    axis=AX.X,
                )
                nc.sync.dma_start(
                    out=mx[b].rearrange("(c p) -> p c", p=128), in_=mxcol[:, :]
                )
                del state[b]

            # Software-pipelined schedule: the DVE-heavy mean path of batch b
            # is traced after batch b+1's compute phases so the DVE stream
            # doesn't stall the next batch's PE/ACT start.
            proj(0)
            for b in range(NB):
                ph1(b)
                if b + 1 < NB:
                    proj(b + 1)
                ph2(b)
                if b > 0:
                    ph3(b - 1)
                ph2b(b)
            ph3(NB - 1)

    nc.compile()
    return nc


def _prep(queries, keys, values, key_mask, wq, bq, wk, bk, wv, bv, wo, bo):
    """Host-side marshalling: shard over batch, transpose to [d, seq] layout,
    compact keys/values by the mask, build the fused weight/selector tensors."""
    q = np.ascontiguousarray(np.asarray(queries, np.float32))
    k = np.ascontiguousarray(np.asarray(keys, np.float32))
    v = np.ascontiguousarray(np.asarray(values, np.float32))
    mask = np.asarray(key_mask).astype(bool)

    counts = mask.sum(1)
    kmax = int(counts.max())
    nk = max(1, (kmax + 127) // 128)
    KP = 128 * nk

    kc = np.zeros((B, D_MODEL, KP), np.float32)
    vc = np.zeros((B, D_MODEL, KP), np.float32)
    mn = np.full((B, KP), NEG, np.float32)
    for b in range(B):
        idx = np.nonzero(mask[b])[0]
        c = len(idx)
        if c:
            kc[b, :, :c] = k[b, idx].T
            vc[b, :, :c] = v[b, idx].T
            mn[b, :c] = 0.0

    def _with_ones(x, nzero=0):  # [B, 64, S] -> [B, 65+nzero, S]
        ones = np.ones((B, 1, x.shape[2]), np.float32)
        parts = [x, ones]
        if nzero:
            parts.append(np.zeros((B, nzero, x.shape[2]), np.float32))
        return np.ascontiguousarray(np.concatenate(parts, axis=1))

    qT = _with_ones(q.transpose(0, 2, 1))
    kT = _with_ones(kc)
    vT = _with_ones(vc)
    mneg = np.ascontiguousarray(mn.reshape(B, nk, 128).transpose(0, 2, 1))

    wq = np.asarray(wq, np.float32); bq = np.asarray(bq, np.float32)
    wk = np.asarray(wk, np.float32); bk = np.asarray(bk, np.float32)
    wv = np.asarray(wv, np.float32); bv = np.asarray(bv, np.float32)
    wo = np.asarray(wo, np.float32); bo = np.asarray(bo, np.float32)

    wqb = np.concatenate([wq, bq[None]], 0).astype(np.float32)
    wkb = np.concatenate([wk, bk[None]], 0).astype(np.float32)
    wvo = np.zeros((65, 68), np.float32)
    for h in range(H):
        wvo[:64, 17 * h : 17 * h + 16] = wv[:, 16 * h : 16 * h + 16]
        wvo[64, 17 * h : 17 * h + 16] = bv[16 * h : 16 * h + 16]
        wvo[64, 17 * h + 16] = 1.0
    wob = np.zeros((128, 64), np.float32)
    for h in range(H):
        wob[32 * h : 32 * h + 16] = 4.0 * wo[16 * h : 16 * h + 16]
    selm = np.zeros((128, 4), np.float32)
    for h in range(H):
        selm[32 * h + 16, h] = 1.0
    bselm = np.zeros((4, 512), np.float32)
    for h in range(H):
        bselm[h, 128 * h : 128 * (h + 1)] = 1.0
    selbm = np.zeros((4, 128), np.float32)
    for h in range(H):
        selbm[h, 32 * h : 32 * h + 16] = 1.0

    import ml_dtypes
    bfd = ml_dtypes.bfloat16
    qT = qT.astype(bfd); kT = kT.astype(bfd); vT = vT.astype(bfd)
    bo65 = np.zeros((65, 64), np.float32)
    bo65[64] = bo
    cpk = np.zeros((128, 1096), np.float32)
    cpk[0:65, 0:64] = wqb
    cpk[0:65, 64:128] = wkb
    cpk[0:65, 128:196] = wvo
    cpk[:, 196:260] = wob
    cpk[0:65, 260:324] = bo65
    cpk[:, 324:328] = selm
    cpk[0:4, 328:840] = bselm
    cpk[0:4, 840:968] = selbm
    cpk[:, 968:1096] = np.eye(128)
    shared = dict(cpack=cpk.astype(bfd))
    in_maps = []
    for i in range(NCORES):
        s = slice(NB * i, NB * (i + 1))
        m = dict(qT=qT[s], kT=kT[s], vT=vT[s], mneg=mneg[s], **shared)
        in_maps.append(m)
    return nk, in_maps


def _ensure_ntff_hook():
    """The agent image's antenv lacks axon_hooks; synthesize it so
    run_bass_kernel_spmd(trace=True) can NTFF-profile via libaxon_pjrt.so."""
    import sys as _sys
    import types
    try:
        from antenv.axon_hooks import get_axon_ntff_profile_hook  # noqa: F401
        return
    except ImportError:
        pass
    mod = types.ModuleType("antenv.axon_hooks")
    _state = {"hook": None}
    mod.set_axon_ntff_profile_hook = lambda h: _state.__setitem__("hook", h)
    mod.get_axon_ntff_profile_hook = lambda: _state["hook"]
    _sys.modules["antenv.axon_hooks"] = mod
    import antenv
    antenv.axon_hooks = mod
    if "/root/.axon_site/trn_agent_boot" not in _sys.path:
        _sys.path.insert(0, "/root/.axon_site/trn_agent_boot")
    try:
        import trn_boot
        hook = trn_boot._ntff_profile_via_ctypes("/opt/axon/libaxon_pjrt.so")
        mod.set_axon_ntff_profile_hook(hook)
    except Exception as e:  # degrade: tracing skipped
        print(f"ntff hook setup failed: {e}", file=sys.stderr)


def _run(in_maps, nk, trace=False):
    global LAST_EXEC_NS, LAST_TRACE_DIR, LAST_RESULT
    from concourse.bass_utils import run_bass_kernel_spmd

    if trace:
        _ensure_ntff_hook()

    if nk not in _CACHE:
        _CACHE[nk] = _build(nk)
    nc = _CACHE[nk]
    res = run_bass_kernel_spmd(nc, in_maps, core_ids=list(range(NCORES)), trace=trace)
    LAST_RESULT = res
    if res.exec_time_ns is not None:
        LAST_EXEC_NS = res.exec_time_ns
    return res.results


def kernel(queries, keys, values, key_mask, wq, bq, wk, bk, wv, bv, wo, bo,
           _trace=False):
    nk, in_maps = _prep(queries, keys, values, key_mask,
                        wq, bq, wk, bk, wv, bv, wo, bo)
    results = _run(in_maps, nk, trace=_trace)
    attended = np.zeros((B, N, D_MODEL), np.float32)
    max_attn = np.zeros((B, N), np.float32)
    for i in range(NCORES):
        attended[NB * i : NB * (i + 1)] = np.asarray(results[i]["att"])
        max_attn[NB * i : NB * (i + 1)] = np.asarray(results[i]["mx"])
    # All-masked batches divide by a zero denominator on device; the
    # reference defines their outputs as bias / zeros. Patch on host.
    dead = ~np.asarray(key_mask).astype(bool).any(axis=1)
    if dead.any():
        attended[dead] = np.asarray(bo, np.float32)[None, None, :]
        max_attn[dead] = 0.0
    return attended, max_attn
